# revision 1
# baseline (speedup 1.0000x reference)
"""CTC batch loss (Keras convention, blank = C-1) on 8 Trainium2 NeuronCores.

Strategy (pure data parallel, 128 examples per core = 128 SBUF partitions):
  * Prob-domain scaled forward DP (mathematically identical to the reference's
    log-space DP, including the exact log(p + 1e-7) epsilon, which is folded
    into the gather as E = onehot + eps so gathered values are p + eps).
  * Label gather via per-example one-hot matmul on the TensorEngine:
    bf16 pipeline: gpsimd cast-DMA (f32->bf16 inline), XBAR dma transpose
    (t,c)->(c,t), matmul E'^T . X^T -> PSUM f32 [65, t], DVE escape copy,
    DMA re-layout into batch-partitioned p_store (f32).
  * Serial DP over T: 4 DVE ops/step with guard columns making the s-1/s-2
    shifts plain AP offsets; the skip-transition mask runs on GPSIMD off the
    critical path; rescale every 8 steps, log-corrections collected in a
    strip and reduced once at the end.
"""

import sys
from contextlib import ExitStack

import numpy as np

for _p in ("/opt/trn_rl_repo",):
    if _p not in sys.path:
        sys.path.insert(0, _p)

import concourse.bass as bass
import concourse.tile as tile
from concourse import mybir
from concourse.bass_utils import run_bass_kernel_spmd

# Problem constants (hardcoded per spec nn_CTC_55808805045003)
B, T, C, L = 1024, 256, 128, 64
NCORES = 8
BL = B // NCORES          # 128 examples per core
S = 2 * L + 1             # 129 extended labels
NS = L + 1                # 65 gather columns (64 labels + blank)
EPS = 1e-7
CH = 128                  # time chunk
NCH = T // CH             # 2
RESC = 8                  # rescale period
GRP = 16                  # examples per cast-DMA group

f32 = mybir.dt.float32
bf16 = mybir.dt.bfloat16
f16 = mybir.dt.float16
i32 = mybir.dt.int32

# gather-pipeline storage dtype for probabilities (bf16 or f16).
# f16 has a 10-bit mantissa (4x finer than bf16); p < 6e-5 lands in f16
# subnormals, which numpy handles exactly and HW needs to not flush.
import os as _os

GDT_NAME = _os.environ.get("CTC_GDT", "f16")
GDT = {"bf16": bf16, "f16": f16}[GDT_NAME]
GDT_NP = {"bf16": None, "f16": np.float16}[GDT_NAME]
ADD = mybir.AluOpType.add
MULT = mybir.AluOpType.mult
ISEQ = mybir.AluOpType.is_equal
NEQ = mybir.AluOpType.not_equal
AX_X = mybir.AxisListType.X
AFT = mybir.ActivationFunctionType


def _body(tc, loss_ap, yp, lab_ap, e_ap):
    nc = tc.nc
    with ExitStack() as ctx:
        const = ctx.enter_context(tc.tile_pool(name="const", bufs=1))
        dstage = ctx.enter_context(tc.tile_pool(name="dstage", bufs=1, space="DRAM"))
        xtp = ctx.enter_context(tc.tile_pool(name="xt", bufs=8))
        gps = ctx.enter_context(tc.tile_pool(name="gpsum", bufs=2, space="PSUM"))
        gsb = ctx.enter_context(tc.tile_pool(name="gsb", bufs=2))
        tiny = ctx.enter_context(tc.tile_pool(name="tiny", bufs=6))

        # ---- label-derived constants (host-computed, DMA'd in) ----
        Eall = const.tile([128, BL * NS], GDT)
        nc.sync.dma_start(Eall[:], e_ap[:, :])
        m_odd = const.tile([128, L], f32)
        nc.sync.dma_start(m_odd[:], lab_ap[:, :])

        # per-chunk gathered probs: p_store[ch][b, s*CH + t] = p(b, ch*CH+t, ext65[b,s]) + eps
        p_stores = [
            const.tile([128, NS * CH], f32, name=f"p_store{ch}") for ch in range(NCH)
        ]
        ps3 = [
            p_stores[ch][:].rearrange("p (s t) -> p s t", s=NS) for ch in range(NCH)
        ]

        ystages = [
            dstage.tile([BL, CH, C], GDT, name=f"ystage{ch}") for ch in range(NCH)
        ]

        GT = 8  # examples per XBAR transpose / per PSUM+gs tile / per p_store DMA

        def gather_chunk(ch):
            t0 = ch * CH
            ystage = ystages[ch]
            for g in range(BL // GRP):
                # f32 -> f16 cast inline in the DMA (SWDGE), DRAM -> DRAM
                nc.gpsimd.dma_start(
                    ystage[g * GRP : (g + 1) * GRP, :, :],
                    yp[g * GRP : (g + 1) * GRP, t0 : t0 + CH, :],
                )
            for g in range(BL // GT):
                b0 = g * GT
                # one XBAR transpose covers GT examples: [GT*CH, C] -> [C, GT*CH]
                xt = xtp.tile([C, GT * CH], GDT)
                nc.sync.dma_start_transpose(
                    xt[:],
                    ystage[b0 : b0 + GT, :, :].rearrange("b t c -> (b t) c"),
                )
                gp = gps.tile([NS, GT * CH], f32)
                for i in range(GT):
                    b = b0 + i
                    nc.tensor.matmul(
                        gp[:, i * CH : (i + 1) * CH],
                        Eall[:, b * NS : (b + 1) * NS],
                        xt[:, i * CH : (i + 1) * CH],
                        start=True,
                        stop=True,
                    )
                gs = gsb.tile([NS, GT * CH], f32)
                nc.vector.tensor_copy(gs[:], gp[:])
                # per-example re-layout into batch partitions, spread across
                # both HWDGE queues (SP + ACT)
                for i in range(GT):
                    b = b0 + i
                    eng = nc.scalar if (b % 2) else nc.sync
                    eng.dma_start(
                        ps3[ch][b : b + 1, :, :], gs[:, i * CH : (i + 1) * CH]
                    )

        for ch in range(NCH):
            gather_chunk(ch)

        # ---- DP state ----
        # alpha cols: 0,1 = zero guards; 2..130 = s=0..128; 131 pad
        alpha = const.tile([128, 132], f32)
        u = const.tile([128, 132], f32)
        v_odd = const.tile([128, 64], f32)
        aM = const.tile([128, 66], f32)  # col 0 guard; 1..64 = masked odd alphas
        strip = const.tile([128, 32], f32)

        nc.vector.memset(alpha[:], 0.0)
        nc.vector.memset(aM[:], 0.0)

        # t = 0 init: alpha[s=0] = p_blank(t=0), alpha[s=1] = p_lab0(t=0)
        nc.vector.tensor_copy(alpha[:, 2:3], ps3[0][:, NS - 1 : NS, 0:1].squeeze(2))
        nc.vector.tensor_copy(alpha[:, 3:4], ps3[0][:, 0:1, 0:1].squeeze(2))
        # aM[1+j'] = alpha_odd[j'] * m_dest[j'], m_dest[j'] = (lab[j'+1] != lab[j'])
        nc.gpsimd.tensor_tensor(aM[:, 1:2], alpha[:, 3:4], m_odd[:, 0:1], MULT)

        # running rescale factor, applied inside opC's scalar slot; 1.0 except
        # on the step right after each row-sum snapshot
        r_ap = const.tile([128, 1], f32)
        nc.vector.memset(r_ap[:], 1.0)

        k_resc = 0
        for t in range(1, T):
            p3 = ps3[t // CH]
            tt = t % CH
            p_lab = p3[:, 0:64, tt : tt + 1].squeeze(2)
            p_bl = p3[:, 64:65, tt : tt + 1]
            snap = t % RESC == 0  # snapshot row-sum this step, rescale next step
            # u[s] = alpha[s] + alpha[s-1]
            nc.vector.tensor_tensor(u[:, 2:131], alpha[:, 2:131], alpha[:, 1:130], ADD)
            # v_odd[j] = u[2j+3] + aM_prev[j-1]
            nc.vector.tensor_tensor(v_odd[:], u[:, 3:130:2], aM[:, 0:64], ADD)
            # alpha_odd = (v_odd * r) * p_lab
            nc.vector.scalar_tensor_tensor(
                alpha[:, 3:130:2], v_odd[:], r_ap[:], p_lab, MULT, MULT
            )
            # alpha_even = (u_even * r) * p_blank
            nc.vector.tensor_scalar(
                alpha[:, 2:131:2], u[:, 2:131:2], r_ap[:], p_bl, MULT, MULT
            )
            # masked odd alphas for the next step's skip term (off critical path);
            # source j'=0..62 feeds destination j'+1, gated by m_dest[j']
            nc.gpsimd.tensor_tensor(
                aM[:, 1:64], alpha[:, 3:128:2], m_odd[:, 0:63], MULT
            )
            if snap:
                cs = tiny.tile([128, 1], f32)
                nc.vector.tensor_reduce(cs[:], alpha[:, 2:131], AX_X, ADD)
                nc.vector.reciprocal(r_ap[:], cs[:])
                nc.scalar.activation(strip[:, k_resc : k_resc + 1], cs[:], AFT.Ln)
                k_resc += 1
            elif t % RESC == 1 and t > 1:
                # r was consumed by this step's opC ops; reset to 1.0
                nc.gpsimd.memset(r_ap[:], 1.0)

        # loss = -(sum_k log c_k + log(alpha[S-1] + alpha[S-2]))
        lik = tiny.tile([128, 1], f32)
        nc.vector.tensor_tensor(lik[:], alpha[:, 129:130], alpha[:, 130:131], ADD)
        nc.scalar.activation(strip[:, 31:32], lik[:], AFT.Ln)
        assert k_resc == 31
        slog = tiny.tile([128, 1], f32)
        nc.vector.tensor_reduce(slog[:], strip[:], AX_X, ADD)
        lout = tiny.tile([128, 1], f32)
        nc.vector.tensor_scalar(lout[:], slog[:], -1.0, None, MULT)
        nc.sync.dma_start(loss_ap[:, :], lout[:])


def build_nc():
    nc = bass.Bass("TRN2", target_bir_lowering=False, debug=False)
    yp = nc.dram_tensor("y_pred", [BL, T, C], f32, kind="ExternalInput").ap()
    lab = nc.dram_tensor("m_odd", [BL, L], f32, kind="ExternalInput").ap()
    e_in = nc.dram_tensor("e_all", [128, BL * NS], GDT, kind="ExternalInput").ap()
    loss = nc.dram_tensor("loss", [BL, 1], f32, kind="ExternalOutput").ap()
    with tile.TileContext(nc) as tc:
        _body(tc, loss, yp, lab, e_in)
    return nc


def host_label_consts(y_true):
    """E' one-hot (+eps, bf16) and skip-mask, per core: pure functions of labels."""
    import ml_dtypes

    lab = np.asarray(y_true).astype(np.int64)  # [B, L]
    outs = []
    for i in range(NCORES):
        lb = lab[i * BL : (i + 1) * BL]  # [128, 64]
        ext = np.concatenate(
            [lb, np.full((BL, 1), C - 1, np.int64)], axis=1
        )  # [128, 65]
        e = (np.arange(128)[:, None, None] == ext[None, :, :]).astype(np.float32)
        npdt = GDT_NP or ml_dtypes.bfloat16
        e = (e + EPS).astype(npdt).reshape(128, BL * NS)
        # destination-indexed skip mask: m[j'] = (lab[j'+1] != lab[j']), j'=0..62
        m = np.zeros((BL, L), np.float32)
        m[:, 0:63] = (lb[:, 1:] != lb[:, :-1]).astype(np.float32)
        outs.append((e, m))
    return outs


_CACHE = {}

# --- BIR legalizer -----------------------------------------------------------
# This container's walrus encodes at most ONE sync wait on SP-queue
# instruction classes (PSEUDO_DMA_DIRECT2D / XPOSE / CTRL): "Too many sync
# wait commands". Tile freely emits >=2 waits per instruction. Split the
# extras onto NoOps inserted just before (same engine stream => semantics
# preserved, waits satisfied in order).
_SPLIT_OPS = {"DMACopy", "DmaTransposeAnt", "DMAGatherAnt", "Drain", "NoOp"}


def _legalize_bir(bir_bytes):
    import orjson

    d = orjson.loads(bir_bytes)
    n_new = 0
    for fn in d.get("functions", []):
        for blk in fn.get("blocks", []):
            insts = blk.get("instructions")
            if not insts:
                continue
            out = []
            for ins in insts:
                si = ins.get("sync_info")
                if si:
                    waits = si.get("on_wait") or []
                    if len(waits) > 1:
                        for w in waits[:-1]:
                            n_new += 1
                            out.append(
                                {
                                    "debug": ins.get("debug", 0),
                                    "engine": ins["engine"],
                                    "ins": [],
                                    "outs": [],
                                    "name": f"ZW-{n_new}",
                                    "opcode": "NoOp",
                                    "sync_info": {"on_wait": [w], "on_update": []},
                                }
                            )
                        si["on_wait"] = [waits[-1]]
                out.append(ins)
            blk["instructions"] = out
    return orjson.dumps(d)


def _install_bir_legalizer():
    import concourse.bass2jax as b2j

    if getattr(b2j, "_ctc_legalizer_installed", False):
        return
    orig = b2j.compile_bir_kernel

    def wrapper(bir_json, tmpdir, neff_name="file.neff"):
        bir_json = _legalize_bir(bir_json)
        return orig(bir_json, tmpdir, neff_name=neff_name)

    b2j.compile_bir_kernel = wrapper
    b2j._ctc_legalizer_installed = True


def kernel(y_true, y_pred):
    assert y_pred.shape == (B, T, C) and y_true.shape == (B, L)
    _install_bir_legalizer()
    nc = _CACHE.get("nc")
    if nc is None:
        nc = _CACHE["nc"] = build_nc()
    yp = np.ascontiguousarray(y_pred, dtype=np.float32)
    consts = host_label_consts(y_true)
    in_maps = [
        {
            "y_pred": yp[i * BL : (i + 1) * BL],
            "m_odd": consts[i][1],
            "e_all": consts[i][0],
        }
        for i in range(NCORES)
    ]
    res = run_bass_kernel_spmd(nc, in_maps, list(range(NCORES)))
    out = np.concatenate([res.results[i]["loss"] for i in range(NCORES)], axis=0)
    return out.astype(np.float32)



# revision 14
# speedup vs baseline: 1.5300x; 1.5300x over previous
"""CTC batch loss (Keras convention, blank = C-1) on 8 Trainium2 NeuronCores.

Strategy (pure data parallel, 128 examples per core = 128 SBUF partitions):
  * Prob-domain scaled DP, split forward/backward: fwd runs t=0..127 from the
    start, bwd runs t=255..128 from the end (reversed state order so both
    recurrences shift the same direction); likelihood = (A alpha_127) . gamma_128.
    127 rounds instead of 255, with each round's ops covering both halves.
  * Label gather via per-example one-hot matmul on the TensorEngine:
    SWDGE cast-DMA f32->bf16 (DRAM->DRAM), XBAR transpose (t,c)->(c,t) split
    over the SP/ACT queues, matmul E_b^T . X^T -> PSUM [64, 256], escape
    copies on DVE/ACT, batched SWDGE re-layout into batch-partitioned p_store.
  * p values pre-shuffled (on Pool, in round-ranges so the DP can start after
    the first range) into round-major p_odd_cat / pm_cat (mask pre-folded), so
    the steady-state round is 5 TT + 2 TS, all contiguous bf16 (2x/4x DVE
    modes), zero cross-engine dependencies.  Rescale every 8 rounds scales the
    state tiles directly; log corrections collected in a strip, reduced once.
"""

import sys
from contextlib import ExitStack

import numpy as np

for _p in ("/opt/trn_rl_repo",):
    if _p not in sys.path:
        sys.path.insert(0, _p)

import concourse.bass as bass
import concourse.tile as tile
from concourse import mybir
from concourse.bass_utils import run_bass_kernel_spmd

# Problem constants (hardcoded per spec nn_CTC_55808805045003)
B, T, C, L = 1024, 256, 128, 64
NCORES = 8
BL = B // NCORES          # 128 examples per core
S = 2 * L + 1             # 129 extended labels
EPS = 1e-7
NR = T // 2               # 128 DP rounds (round 0 = init)
W = 130                   # p_odd_cat row width: 64 fwd + 2 garbage + 64 bwd
GT = 8                    # examples per XBAR transpose
GR = 16                   # examples per batched SWDGE relayout

f32 = mybir.dt.float32
bf16 = mybir.dt.bfloat16
i32 = mybir.dt.int32

ADD = mybir.AluOpType.add
MULT = mybir.AluOpType.mult
AX_X = mybir.AxisListType.X
AFT = mybir.ActivationFunctionType

RESC = 8                  # rescale period (rounds)


def _body(tc, loss_ap, yp, e_ap, mcat_ap, dumps=None):
    nc = tc.nc
    with ExitStack() as ctx:
        const = ctx.enter_context(tc.tile_pool(name="const", bufs=1))
        dstage = ctx.enter_context(tc.tile_pool(name="dstage", bufs=1, space="DRAM"))
        xtp = ctx.enter_context(tc.tile_pool(name="xt", bufs=3))
        escp = ctx.enter_context(tc.tile_pool(name="esc", bufs=2))
        gps = ctx.enter_context(tc.tile_pool(name="gpsum", bufs=6, space="PSUM"))
        tiny = ctx.enter_context(tc.tile_pool(name="tiny", bufs=1))

        ystage = dstage.tile([BL, T, C], bf16)
        p_dram = dstage.tile([BL, 65 * T], bf16, name="p_dram")

        # ---- persistent tiles ----
        E = const.tile([128, BL * 65], bf16)          # one-hot+eps, 64 labels+blank
        mcat = const.tile([128, W], bf16)             # skip masks, cat layout
        p_store = const.tile([128, 65 * T], bf16)     # [b, s*256 + t], s=64 blank
        p_blf = const.tile([128, T], f32)
        POC = const.tile([128, NR * W], bf16)         # p_odd_cat, round-major
        PM = const.tile([128, NR * W], bf16)          # mask * p_odd_cat
        MREP = const.tile([128, NR * W], bf16)        # mcat replicated per round

        # host constants in via SWDGE (spreads across all 16 DMA engines)
        nc.gpsimd.dma_start(E[:], e_ap[:, :])
        nc.gpsimd.dma_start(mcat[:], mcat_ap[:, :])

        # ---- gather phase ----
        # 1) f32 -> bf16 cast inline in the DMA (SWDGE), DRAM -> DRAM
        for g in range(BL // GT):
            nc.gpsimd.dma_start(
                ystage[g * GT : (g + 1) * GT, :, :],
                yp[g * GT : (g + 1) * GT, :, :],
            )

        # m_rep: replicate mcat across all rounds by doubling (DVE, no deps on
        # gather data, runs immediately)
        nc.vector.tensor_copy(MREP[:, 0:W], mcat[:])
        n = W
        while n < NR * W:
            m = min(n, NR * W - n)
            nc.vector.tensor_copy(MREP[:, n : n + m], MREP[:, 0:m])
            n += m

        escs = [None] * (BL // GR)

        for g in range(BL // GT):
            b0 = g * GT
            # XBAR transpose: [GT*T, C] -> [C, GT*T]; alternate SP/ACT queues
            xt = xtp.tile([128, GT * T], bf16)
            qeng = nc.sync if (g % 2 == 0) else nc.scalar
            qeng.dma_start_transpose(
                xt[:], ystage[b0 : b0 + GT, :, :].rearrange("b t c -> (b t) c")
            )
            rg = g // 2  # relayout group of GR examples
            if g % 2 == 0:
                escs[rg] = escp.tile([65, GR * T], bf16, name="esc")
            esc = escs[rg]
            eoff = (g % 2) * GT
            for i in range(GT):
                b = b0 + i
                gp = gps.tile([65, T], f32)
                nc.tensor.matmul(
                    gp[:, :],
                    E[:, b * 65 : (b + 1) * 65],
                    xt[:, i * T : (i + 1) * T],
                    start=True,
                    stop=True,
                )
                ecol = (eoff + i) * T
                if b % 2 == 0:
                    nc.vector.tensor_copy(esc[:, ecol : ecol + T], gp[:, :])
                else:
                    nc.scalar.copy(esc[:, ecol : ecol + T], gp[:, :])
            if g % 2 == 1:
                # batched re-layout via DRAM bounce (partition dim must lead
                # SBUF DMA APs, so an SBUF->SBUF batch transpose is illegal):
                # esc [65, GR*T] -> p_dram[b, s*T + t], iterated (s, e, t)
                nc.gpsimd.dma_start(
                    p_dram[rg * GR : (rg + 1) * GR, :].rearrange(
                        "e (s t) -> s e t", s=65
                    ),
                    esc[:].rearrange("s (e t) -> s e t", e=GR),
                )

        # one contiguous load DRAM -> SBUF batch-partitioned p_store
        nc.gpsimd.dma_start(p_store[:], p_dram[:, :])

        # blank probs (p_store row s=64) to f32 for the TS scalar slots
        nc.vector.tensor_copy(p_blf[:], p_store[:, 64 * T : 65 * T])

        # ---- p shuffles into round-major layout (Pool, 4 ranges) ----
        ps_ts = p_store[:].rearrange("p (s t) -> p t s", s=65)
        poc3 = POC[:].rearrange("p (r c) -> p r c", c=W)
        # garbage columns 64, 65 stay zero
        nc.vector.memset(poc3[:, :, 64:66], 0.0)
        NRANGE = 4
        RW = NR // NRANGE
        for k in range(NRANGE):
            r0, r1 = k * RW, (k + 1) * RW
            # fwd: POC[., rho, j] = p_store[., j, t=rho]
            nc.gpsimd.tensor_copy(poc3[:, r0:r1, 0:64], ps_ts[:, r0:r1, 0:64])
            # bwd: POC[., rho, 66+m] = p_store[., 63-m, t=255-rho]
            nc.gpsimd.tensor_copy(
                poc3[:, r0:r1, 66:W],
                ps_ts[:, 255 - r0 : 255 - r1 : -1, 63::-1],
            )
            # mask fold: PM = POC * m_rep  (garbage cols -> 0)
            nc.gpsimd.tensor_tensor(
                PM[:, r0 * W : r1 * W],
                POC[:, r0 * W : r1 * W],
                MREP[:, r0 * W : r1 * W],
                MULT,
            )

        # ---- DP state ----
        # AO: odd states. col 0 guard, 1..64 fwd j=0..63 (s=2j+1),
        #     65..66 guard/garbage, 67..130 bwd m=0..63 (nu=2m+1).
        # AE: even states. col 0 guard, 1..65 fwd i=0..64 (s=2i),
        #     66 garbage, 67..131 bwd m'=0..64 (nu=2m').
        AO = const.tile([128, 131], bf16)
        AE = const.tile([128, 132], bf16)
        U1 = const.tile([128, 130], bf16)
        U2 = const.tile([128, 131], bf16)
        Z = const.tile([128, 130], bf16)
        Q = const.tile([128, 130], bf16)
        strip = const.tile([128, 18], f32)
        nc.vector.memset(AO[:], 0.0)
        nc.vector.memset(AE[:], 0.0)

        # init: alpha_0 at t=0: s=0 (blank) -> AE[1], s=1 (lab0) -> AO[1]
        nc.vector.tensor_copy(AE[:, 1:2], p_blf[:, 0:1])
        nc.vector.tensor_copy(AO[:, 1:2], p_store[:, 0:1])
        # init: gamma_255 at t=255: s=128 (blank) -> AE[67], s=127 -> AO[67]
        nc.vector.tensor_copy(AE[:, 67:68], p_blf[:, 255:256])
        nc.vector.tensor_copy(AO[:, 67:68], p_store[:, 63 * T + 255 : 63 * T + 256])

        cs1 = tiny.tile([128, 1], f32)
        cs2 = tiny.tile([128, 1], f32)
        cs = tiny.tile([128, 1], f32)
        css = tiny.tile([128, 1], f32)
        r_ap = tiny.tile([128, 1], f32)

        k_resc = 0
        for rho in range(1, NR):
            base = rho * W
            # u_odd[idx] = AO[1+idx] + AE[1+idx], idx=0..129
            nc.vector.tensor_tensor(U1[:], AO[:, 1:131], AE[:, 1:131], ADD)
            # u_even[idx] = AE[1+idx] + AO[idx], idx=0..130
            nc.vector.tensor_tensor(U2[:], AE[:, 1:132], AO[:, 0:131], ADD)
            # z = u_odd * p_odd ; q = alpha_odd(shifted) * (m*p_odd)
            nc.vector.tensor_tensor(Z[:], U1[:], POC[:, base : base + W], MULT)
            nc.vector.tensor_tensor(Q[:], AO[:, 0:130], PM[:, base : base + W], MULT)
            nc.vector.tensor_tensor(AO[:, 1:131], Z[:], Q[:], ADD)
            # evens: scale by blank prob (fwd t=rho, bwd t=255-rho)
            nc.vector.tensor_scalar(
                AE[:, 1:67], U2[:, 0:66], p_blf[:, rho : rho + 1], None, MULT
            )
            nc.vector.tensor_scalar(
                AE[:, 67:132], U2[:, 66:131], p_blf[:, 255 - rho : 256 - rho], None, MULT
            )
            if (rho % RESC == 0 and rho + 1 < NR) or rho in (124, 127):
                # rescale both halves by the shared total; log corr to strip
                nc.vector.tensor_reduce(cs1[:], AO[:, 0:131], AX_X, ADD)
                nc.vector.tensor_reduce(cs2[:], AE[:, 0:132], AX_X, ADD)
                nc.vector.tensor_tensor(cs[:], cs1[:], cs2[:], ADD)
                nc.vector.reciprocal(r_ap[:], cs[:])
                nc.vector.tensor_scalar(AO[:], AO[:], r_ap[:], None, MULT)
                nc.vector.tensor_scalar(AE[:], AE[:], r_ap[:], None, MULT)
                # pre-scale cs by 2^32 (exact): ACT Ln is accurate only down
                # to ~1e-16; corrected by a constant in the final combine
                nc.vector.tensor_scalar(css[:], cs[:], float(2.0 ** 32), None, MULT)
                nc.scalar.activation(strip[:, k_resc : k_resc + 1], css[:], AFT.Ln)
                k_resc += 1
        assert k_resc == 17

        # ---- endgame: L = (A alpha_127) . gamma_128 ----
        # Late rescales (rho=124, 127) keep the cross products ~e^-44, well
        # inside f32/bf16 range; lik is pre-scaled by 2^64 (exact) so the ACT
        # Ln input lands near 1 (the Ln table is inaccurate below ~1e-16).
        UF = tiny.tile([128, 64], bf16)
        QF = tiny.tile([128, 64], bf16)
        VF = tiny.tile([128, 64], bf16)
        UE = tiny.tile([128, 65], bf16)
        D = tiny.tile([128, 129], bf16)
        nc.vector.tensor_tensor(UF[:], AO[:, 1:65], AE[:, 1:65], ADD)
        nc.vector.tensor_tensor(QF[:], AO[:, 0:64], mcat[:, 0:64], MULT)
        nc.vector.tensor_tensor(VF[:], UF[:], QF[:], ADD)
        nc.vector.tensor_tensor(UE[:], AE[:, 1:66], AO[:, 0:65], ADD)
        nc.vector.tensor_tensor(D[:, 0:64], VF[:], AO[:, 130:66:-1], MULT)
        nc.vector.tensor_tensor(D[:, 64:129], UE[:], AE[:, 131:66:-1], MULT)
        lik = tiny.tile([128, 1], f32)
        nc.vector.tensor_reduce(lik[:], D[:], AX_X, ADD)
        lik2 = tiny.tile([128, 1], f32)
        nc.vector.tensor_scalar(lik2[:], lik[:], float(2.0 ** 64), None, MULT)
        lnlik = tiny.tile([128, 1], f32)
        nc.scalar.activation(lnlik[:], lik2[:], AFT.Ln)
        ssum = tiny.tile([128, 1], f32)
        nc.vector.tensor_reduce(ssum[:], strip[:, 0:17], AX_X, ADD)
        # loss = -(ln lik2 - 64 ln 2 + 2 * (sum strip - 17*32 ln 2))
        CADD = float((64 + 2 * 17 * 32) * np.log(2.0))
        t1 = tiny.tile([128, 1], f32)
        nc.vector.tensor_scalar(t1[:], ssum[:], -2.0, CADD, MULT, ADD)
        lout = tiny.tile([128, 1], f32)
        nc.vector.scalar_tensor_tensor(lout[:], lnlik[:], -1.0, t1[:], MULT, ADD)
        nc.sync.dma_start(loss_ap[:, :], lout[:])

        if dumps is not None:
            # debug: convert bf16 tiles to f32 staging and DMA out
            dbg = ctx.enter_context(tc.tile_pool(name="dbg", bufs=1))
            for key, (src_tile, width) in {
                "ps": (p_store, 65 * T),
                "poc": (POC, NR * W),
                "pm": (PM, NR * W),
                "ao": (AO, 131),
                "ae": (AE, 132),
            }.items():
                if key not in dumps:
                    continue
                CHW = 4096
                stg = dbg.tile([128, min(width, CHW)], f32, name="dbgstg")
                off = 0
                while off < width:
                    wdt = min(CHW, width - off)
                    nc.vector.tensor_copy(stg[:, 0:wdt], src_tile[:, off : off + wdt])
                    nc.sync.dma_start(dumps[key][:, off : off + wdt], stg[:, 0:wdt])
                    off += wdt
            if "strip" in dumps:
                nc.sync.dma_start(dumps["strip"][:, :], strip[:])


def build_nc():
    nc = bass.Bass("TRN2", target_bir_lowering=False, debug=False)
    yp = nc.dram_tensor("y_pred", [BL, T, C], f32, kind="ExternalInput").ap()
    e_in = nc.dram_tensor("e_all", [128, BL * 65], bf16, kind="ExternalInput").ap()
    mc_in = nc.dram_tensor("m_cat", [128, W], bf16, kind="ExternalInput").ap()
    loss = nc.dram_tensor("loss", [BL, 1], f32, kind="ExternalOutput").ap()
    with tile.TileContext(nc) as tc:
        _body(tc, loss, yp, e_in, mc_in)
    return nc


def host_label_consts(y_true):
    """E one-hot (+eps, bf16) and cat-layout skip masks: pure label functions."""
    import ml_dtypes

    lab = np.asarray(y_true).astype(np.int64)  # [B, L]
    outs = []
    ar = np.arange(128)
    for i in range(NCORES):
        lb = lab[i * BL : (i + 1) * BL]  # [128, 64]
        # E[c, b*65 + s] = (c == ext[b, s]) + eps, ext = labels then blank
        ext = np.concatenate([lb, np.full((BL, 1), C - 1, np.int64)], axis=1)
        e = (ar[:, None, None] == ext[None, :, :]).astype(np.float32) + EPS
        e = e.astype(ml_dtypes.bfloat16).reshape(128, BL * 65)
        # mcat[b, idx]: idx 0..63 fwd dest j: (lab[j] != lab[j-1]), j>=1
        #              idx 64..65: 0 (garbage)
        #              idx 66+m bwd dest m: (lab[64-m] != lab[63-m]), m>=1
        mc = np.zeros((BL, W), np.float32)
        mc[:, 1:64] = (lb[:, 1:] != lb[:, :-1]).astype(np.float32)
        dif = (lb[:, 1:] != lb[:, :-1]).astype(np.float32)  # [B, 63] at j=1..63
        # bwd m=1..63: mask = dif at position (63-m) i.e. lab[64-m] vs lab[63-m]
        mc[:, 67:130] = dif[:, ::-1]
        outs.append((e, mc.astype(ml_dtypes.bfloat16)))
    return outs


_CACHE = {}

# --- BIR legalizer -----------------------------------------------------------
# This container's walrus encodes at most ONE sync wait on SP-queue
# instruction classes (PSEUDO_DMA_DIRECT2D / XPOSE / CTRL): "Too many sync
# wait commands". Tile freely emits >=2 waits per instruction. Split the
# extras onto NoOps inserted just before (same engine stream => semantics
# preserved, waits satisfied in order).
_SPLIT_OPS = {"DMACopy", "DmaTransposeAnt", "DMAGatherAnt", "Drain", "NoOp"}


def _legalize_bir(bir_bytes):
    import orjson

    d = orjson.loads(bir_bytes)
    n_new = 0
    for fn in d.get("functions", []):
        for blk in fn.get("blocks", []):
            insts = blk.get("instructions")
            if not insts:
                continue
            out = []
            for ins in insts:
                si = ins.get("sync_info")
                if si:
                    waits = si.get("on_wait") or []
                    if len(waits) > 1:
                        for w in waits[:-1]:
                            n_new += 1
                            out.append(
                                {
                                    "debug": ins.get("debug", 0),
                                    "engine": ins["engine"],
                                    "ins": [],
                                    "outs": [],
                                    "name": f"ZW-{n_new}",
                                    "opcode": "NoOp",
                                    "sync_info": {"on_wait": [w], "on_update": []},
                                }
                            )
                        si["on_wait"] = [waits[-1]]
                out.append(ins)
            blk["instructions"] = out
    return orjson.dumps(d)


def _install_bir_legalizer():
    import concourse.bass2jax as b2j

    if getattr(b2j, "_ctc_legalizer_installed", False):
        return
    orig = b2j.compile_bir_kernel

    def wrapper(bir_json, tmpdir, neff_name="file.neff"):
        bir_json = _legalize_bir(bir_json)
        return orig(bir_json, tmpdir, neff_name=neff_name)

    b2j.compile_bir_kernel = wrapper
    b2j._ctc_legalizer_installed = True


def kernel(y_true, y_pred):
    assert y_pred.shape == (B, T, C) and y_true.shape == (B, L)
    _install_bir_legalizer()
    nc = _CACHE.get("nc")
    if nc is None:
        nc = _CACHE["nc"] = build_nc()
    yp = np.ascontiguousarray(y_pred, dtype=np.float32)
    consts = host_label_consts(y_true)
    in_maps = [
        {
            "y_pred": yp[i * BL : (i + 1) * BL],
            "e_all": consts[i][0],
            "m_cat": consts[i][1],
        }
        for i in range(NCORES)
    ]
    res = run_bass_kernel_spmd(nc, in_maps, list(range(NCORES)))
    out = np.concatenate([res.results[i]["loss"] for i in range(NCORES)], axis=0)
    return out.astype(np.float32)
